# revision 20
# baseline (speedup 1.0000x reference)
"""Trainium2 Bass kernel for nn_MCGRU (per-lab GRU over labs, batch-sharded 8 ways).

Math (per reference):
  demo = static @ demo_W.T + demo_b                      [bs, HID]
  xp   = x @ lab_W.T + lab_b                             [bs, T, LAB]
  per-lab GRU over T steps with input size 1, hidden F:
    gi = xp_t[:,:,None]*Wih + bih ; gh = einsum(h,Whh) + bhh
    r = sig(gi_r+gh_r); z = sig(gi_z+gh_z); n = tanh(gi_n + r*gh_n)
    h' = (1-z)*n + z*h
  out = cat(demo, h_T.reshape) @ out_W.T + out_b         [bs, HID]

Device layout (per core, bs_loc=128 batch rows):
  - GRU state h kept as [(lab,f) partitions, batch free]; labs split into two
    groups of 32 => two independent [128,128] chains per core.
  - Gate pre-activations by block-diagonal matmuls; biases applied by a
    leading K=2 selector matmul per psum tile (lab_b folded into gate biases).
  - sigmoid/tanh on ScalarE, gate algebra on VectorE.
All host-side prep is layout-only (transpose/pack/fold of weights).
"""

import ml_dtypes
import numpy as np

BF16 = ml_dtypes.bfloat16
BS, T, LAB, DEMO, HID, F = 1024, 128, 64, 16, 32, 4
NCORES = 8
BSL = BS // NCORES  # 128 batch rows per core
G = 2               # lab groups per core
LPG = LAB // G      # 32 labs per group
TH = T // 2         # t-half length (xp row-stacking)


def _pack_host(inputs):
    """Layout-only host packing of weights + per-core input shards."""
    x = np.asarray(inputs["x"], np.float32)
    static = np.asarray(inputs["static"], np.float32)
    demo_W = np.asarray(inputs["demo_W"], np.float32)
    demo_b = np.asarray(inputs["demo_b"], np.float32)
    lab_W = np.asarray(inputs["lab_W"], np.float32)
    lab_b = np.asarray(inputs["lab_b"], np.float32)
    Wih = np.asarray(inputs["Wih"], np.float32)
    bih = np.asarray(inputs["bih"], np.float32)
    Whh = np.asarray(inputs["Whh"], np.float32)
    bhh = np.asarray(inputs["bhh"], np.float32)
    out_W = np.asarray(inputs["out_W"], np.float32)
    out_b = np.asarray(inputs["out_b"], np.float32)

    shared = {}
    # xp matmul: out[j, n] = sum_l lab_W[j, l] * xT[l, n]
    shared["wlab"] = np.ascontiguousarray(lab_W.T)  # [64, 64]

    # Per-group block-diagonal GRU weights.
    for g in range(G):
        labs = range(g * LPG, (g + 1) * LPG)
        whr = np.zeros((128, 128), np.float32)
        whz = np.zeros((128, 128), np.float32)
        whn = np.zeros((128, 128), np.float32)
        wxr = np.zeros((32, 128), np.float32)
        wxz = np.zeros((32, 128), np.float32)
        wxn = np.zeros((32, 128), np.float32)
        bias4 = np.zeros((4, 128), np.float32)
        for i, l in enumerate(labs):
            s = slice(i * 4, i * 4 + 4)
            # lhsT[k=(i,f_in), m=(i,f_out)] = Whh[l, f_out, f_in]
            whr[s, s] = Whh[l, 0:4, :].T
            whz[s, s] = Whh[l, 4:8, :].T
            whn[s, s] = Whh[l, 8:12, :].T
            wxr[i, s] = Wih[l, 0:4]
            wxz[i, s] = Wih[l, 4:8]
            wxn[i, s] = Wih[l, 8:12]
            # biases; lab_b folded in (xp is computed without lab_b).
            bias4[0, s] = bih[l, 0:4] + bhh[l, 0:4] + Wih[l, 0:4] * lab_b[l]
            bias4[1, s] = bih[l, 4:8] + bhh[l, 4:8] + Wih[l, 4:8] * lab_b[l]
            bias4[2, s] = bhh[l, 8:12]
            bias4[3, s] = bih[l, 8:12] + Wih[l, 8:12] * lab_b[l]
        shared[f"whr{g}"] = whr
        shared[f"whz{g}"] = whz
        shared[f"whn{g}"] = whn
        shared[f"bias4_{g}"] = bias4
        shared[f"_wxr{g}"] = wxr
        shared[f"_wxz{g}"] = wxz
        shared[f"_wxn{g}"] = wxn

    # x-side weights stacked so lhsT slices share the xpa base partition:
    # rows [g*32 : g*32+32] hold group g.
    for nm in ("wxr", "wxz", "wxn"):
        wall = np.zeros((64, 128), np.float32)
        for g in range(G):
            wall[g * 32:(g + 1) * 32, :] = shared[f"_{nm}{g}"]
        shared[f"{nm}a"] = wall
    for g in range(G):
        del shared[f"_wxr{g}"], shared[f"_wxz{g}"], shared[f"_wxn{g}"]

    # selector for the bias matmul: row k -> cols [k*BSL, (k+1)*BSL)
    sel4 = np.zeros((4, 4 * BSL), np.float32)
    for k in range(4):
        sel4[k, k * BSL:(k + 1) * BSL] = 1.0
    shared["sel4"] = sel4

    # Output layer. feat index (l, f) -> col HID + l*4 + f of out_W.
    w_feat = out_W[:, HID:]  # [32, 256]
    for g in range(G):
        wo = np.zeros((128, HID), np.float32)
        for i, l in enumerate(range(g * LPG, (g + 1) * LPG)):
            wo[i * 4:(i + 1) * 4, :] = w_feat[:, l * 4:(l + 1) * 4].T
        shared[f"wout{g}"] = wo
    shared["woutd"] = np.ascontiguousarray(out_W[:, :HID].T)  # [32, 32]
    shared["woutb"] = out_b.reshape(1, HID).copy()            # [1, 32]
    # demo matmul lhsT: [17, 32]; row 0 = demo_b (ones row of statt is row 0)
    wdemo = np.zeros((DEMO + 1, HID), np.float32)
    wdemo[0, :] = demo_b
    wdemo[1:, :] = demo_W.T
    shared["wdemo"] = wdemo

    # Per-core shards. xs [64, T*BSL], col = t*BSL + b.
    xT = np.ascontiguousarray(x.transpose(2, 1, 0))  # [LAB, T, BS]
    in_maps = []
    for c in range(NCORES):
        m = dict(shared)
        xc = xT[:, :, c * BSL:(c + 1) * BSL]  # [64, 128, 128]
        m["xs"] = np.ascontiguousarray(xc.reshape(LAB, T * BSL))
        st = np.ones((DEMO + 1, BSL), np.float32)
        st[1:, :] = static[c * BSL:(c + 1) * BSL, :].T
        m["statt"] = st
        in_maps.append(m)
    # bf16 for matmul operands (PSUM still accumulates fp32)
    bf_names = {"wlab", "wxra", "wxza", "wxna", "sel4"}
    for g in range(G):
        bf_names |= {f"whr{g}", f"whz{g}", f"whn{g}", f"bias4_{g}",
                     f"wout{g}"}
    for m in in_maps:
        for n in list(m):
            if n in bf_names or n == "xs":
                m[n] = m[n].astype(BF16)
        m["woutd"] = m["woutd"].astype(BF16)
    return in_maps


def _build_kernel():
    import concourse.bacc as bacc
    import concourse.tile as tile
    from concourse import mybir
    from concourse._compat import get_trn_type

    f32 = mybir.dt.float32
    bf16 = mybir.dt.bfloat16
    nc = bacc.Bacc(get_trn_type() or "TRN2", target_bir_lowering=False, debug=False)

    # DRAM tensors
    d_xs = nc.dram_tensor("xs", (LAB, T * BSL), bf16, kind="ExternalInput")
    d_st = nc.dram_tensor("statt", (DEMO + 1, BSL), f32, kind="ExternalInput")
    wnames = ["wlab", "sel4", "woutd", "woutb", "wdemo", "wxra", "wxza", "wxna"]
    for g in range(G):
        wnames += [f"whr{g}", f"whz{g}", f"whn{g}", f"bias4_{g}",
                   f"wout{g}"]
    wshapes = {
        "wlab": (LAB, LAB), "sel4": (4, 4 * BSL), "woutd": (HID, HID),
        "woutb": (1, HID), "wdemo": (DEMO + 1, HID),
        "wxra": (64, 128), "wxza": (64, 128), "wxna": (64, 128),
    }
    for g in range(G):
        wshapes.update({
            f"whr{g}": (128, 128), f"whz{g}": (128, 128), f"whn{g}": (128, 128),
            f"bias4_{g}": (4, 128), f"wout{g}": (128, HID),
        })
    bf_set = {"wlab", "wxra", "wxza", "wxna", "sel4", "woutd"}
    for g in range(G):
        bf_set |= {f"whr{g}", f"whz{g}", f"whn{g}", f"bias4_{g}",
                   f"wout{g}"}
    dws = {n: nc.dram_tensor(n, wshapes[n], bf16 if n in bf_set else f32,
                             kind="ExternalInput")
           for n in wnames}
    d_y = nc.dram_tensor("y", (HID, BSL), f32, kind="ExternalOutput")

    Sig = mybir.ActivationFunctionType.Sigmoid
    Tanh = mybir.ActivationFunctionType.Tanh

    with tile.TileContext(nc) as tc:
        with (
            tc.tile_pool(name="const", bufs=1) as cpool,
            tc.tile_pool(name="xp", bufs=1) as xpool,
            tc.tile_pool(name="state", bufs=3) as spool,
            tc.tile_pool(name="work", bufs=4) as wpool,
        ):
            # ---- load weights (small, SWDGE via gpsimd) ----
            wt = {}
            for name in wnames + ["statt"]:
                dt_ = dws[name] if name != "statt" else d_st
                t_ = cpool.tile(list(dt_.shape), dt_.dtype, tag=name)
                nc.gpsimd.dma_start(t_[:], dt_[:])
                wt[name] = t_

            # xp tiles (raw xp, no lab_b): rows = labs, col = t*BSL + b,
            # quartered over t so the scan can start before phase 1 ends.
            QT = T // 4
            xp_q = [xpool.tile([LAB, QT * BSL], bf16, tag=f"xp_sb{q}",
                               name=f"xp_sb{q}")
                    for q in range(4)]

            # ---- phase 1: xp = lab_W @ x (bias folded into gate biases) ----
            with (
                tc.tile_pool(name="xsb", bufs=1) as xsbp,
                tc.tile_pool(name="p1", bufs=3, space="PSUM") as p1pool,
            ):
                xs_q = [xsbp.tile([LAB, T * BSL // 4], bf16, tag=f"xs{q}",
                                  name=f"xs{q}")
                        for q in range(4)]
                for q in range(4):
                    half = T * BSL // 8
                    for j in range(2):
                        cs = slice(j * half, (j + 1) * half)
                        nc.sync.dma_start(xs_q[q][:, cs],
                                          d_xs[:, q * 2 * half + j * half:
                                               q * 2 * half + (j + 1) * half])
                NCH = T * BSL // 512  # 32 chunks of 512
                for i in range(NCH):
                    q, iq = divmod(i, NCH // 4)
                    cs = slice(iq * 512, (iq + 1) * 512)
                    ps = p1pool.tile([LAB, 512], f32, tag="xpp")
                    nc.tensor.matmul(ps[:], wt["wlab"][:], xs_q[q][:, cs],
                                     start=True, stop=True)
                    if i % 2 == 0:
                        nc.vector.tensor_copy(xp_q[q][:, cs], ps[:])
                    else:
                        nc.scalar.copy(xp_q[q][:, cs], ps[:])

            # ---- demo head (independent of scan) ----
            with tc.tile_pool(name="pd", bufs=1, space="PSUM") as pdpool:
                ps_d = pdpool.tile([HID, BSL], f32, tag="psd")
                nc.tensor.matmul(ps_d[:], wt["wdemo"][:], wt["statt"][:],
                                 start=True, stop=True)
                demo_sb = cpool.tile([HID, BSL], bf16, tag="demo_sb")
                nc.vector.tensor_copy(demo_sb[:], ps_d[:])

            # ---- phase 2: GRU scan ----
            h = []
            for g in range(G):
                hg = spool.tile([128, BSL], bf16, tag=f"h{g}")
                nc.gpsimd.memset(hg[:], 0.0)
                h.append(hg)

            with (
                tc.tile_pool(name="prz", bufs=3, space="PSUM") as przp,
            ):
                for t in range(T):
                    q, tq = divmod(t, T // 4)
                    rzs_l, nn_l, tt_l, uu_l, nt_l, zh_l = {}, {}, {}, {}, {}, {}
                    for g in range(G):
                        rsl = slice(g * 32, (g + 1) * 32)
                        xpa = xp_q[q][rsl, tq * BSL:(tq + 1) * BSL]
                        # one psum bank per group: [r | z | gh_n | gi_n]
                        nn = przp.tile([128, 4 * BSL], f32, tag=f"rz{g}")
                        nn_l[g] = nn
                        # Region runs must be consecutive and never revisit
                        # a psum region (HW accumulation constraint).
                        nc.tensor.matmul(nn[:], wt[f"bias4_{g}"][:],
                                         wt["sel4"][:], start=True, stop=False)
                        nc.tensor.matmul(nn[:, 0:BSL], wt[f"whr{g}"][:], h[g][:],
                                         start=False, stop=False)
                        nc.tensor.matmul(nn[:, 0:BSL], wt["wxra"][rsl, :], xpa,
                                         start=False, stop=False)
                        nc.tensor.matmul(nn[:, BSL:2 * BSL], wt[f"whz{g}"][:],
                                         h[g][:], start=False, stop=False)
                        nc.tensor.matmul(nn[:, BSL:2 * BSL], wt["wxza"][rsl, :],
                                         xpa, start=False, stop=False)
                        nc.tensor.matmul(nn[:, 2 * BSL:3 * BSL], wt[f"whn{g}"][:],
                                         h[g][:], start=False, stop=False)
                        nc.tensor.matmul(nn[:, 3 * BSL:], wt["wxna"][rsl, :],
                                         xpa, start=False, stop=True)
                        # sigmoid split per gate: r is on the critical path.
                        rzs = wpool.tile([128, 2 * BSL], bf16, tag=f"rzs{g}")
                        rzs_l[g] = rzs
                        nc.scalar.activation(rzs[:, 0:BSL], nn[:, 0:BSL], Sig)
                        tt = wpool.tile([128, BSL], bf16, tag=f"tt{g}")
                        tt_l[g] = tt
                        nc.vector.tensor_mul(tt[:], rzs[:, 0:BSL],
                                             nn[:, 2 * BSL:3 * BSL])
                        nc.scalar.activation(rzs[:, BSL:], nn[:, BSL:2 * BSL],
                                             Sig)
                    for g in range(G):
                        rzs, nn, tt = rzs_l[g], nn_l[g], tt_l[g]
                        uu = wpool.tile([128, BSL], f32, tag=f"uu{g}")
                        nc.vector.tensor_add(uu[:], tt[:], nn[:, 3 * BSL:])
                        zh = wpool.tile([128, BSL], bf16, tag=f"zh{g}")
                        nc.vector.tensor_mul(zh[:], rzs[:, BSL:], h[g][:])
                        zh_l[g] = zh
                        nt = wpool.tile([128, BSL], bf16, tag=f"nt{g}")
                        nt_l[g] = nt
                        nc.scalar.activation(nt[:], uu[:], Tanh)
                    for g in range(G):
                        rzs, nt, zh = rzs_l[g], nt_l[g], zh_l[g]
                        # h' = z*h + (1-z)*n = zh - (z-1)*n
                        aa = wpool.tile([128, BSL], bf16, tag=f"aa{g}")
                        nc.vector.scalar_tensor_tensor(
                            aa[:], rzs[:, BSL:], 1.0, nt[:],
                            mybir.AluOpType.subtract, mybir.AluOpType.mult)
                        hn = spool.tile([128, BSL], bf16, tag=f"h{g}")
                        nc.vector.tensor_sub(hn[:], zh[:], aa[:])
                        h[g] = hn

            # ---- phase 3: output head ----
            with tc.tile_pool(name="po", bufs=1, space="PSUM") as popool:
                ps_o = popool.tile([HID, BSL], f32, tag="pso")
                nc.tensor.matmul(ps_o[:], wt["wout0"][:], h[0][:],
                                 start=True, stop=False)
                nc.tensor.matmul(ps_o[:], wt["wout1"][:], h[1][:],
                                 start=False, stop=False)
                nc.tensor.matmul(ps_o[:], wt["woutd"][:], demo_sb[:],
                                 start=False, stop=False)
                nc.tensor.matmul(ps_o[:], wt["woutb"][:],
                                 wt["statt"][0:1, :],
                                 start=False, stop=True)
                y_sb = cpool.tile([HID, BSL], f32, tag="y_sb")
                nc.vector.tensor_copy(y_sb[:], ps_o[:])
                nc.sync.dma_start(d_y[:], y_sb[:])

    nc.compile()
    return nc


_NC_CACHE = None


def _get_nc():
    global _NC_CACHE
    if _NC_CACHE is None:
        _NC_CACHE = _build_kernel()
    return _NC_CACHE


def kernel(**inputs):
    from concourse import bass_utils

    in_maps = _pack_host(inputs)
    nc = _get_nc()
    res = bass_utils.run_bass_kernel_spmd(nc, in_maps, list(range(NCORES)))
    ys = [np.asarray(res.results[c]["y"]) for c in range(NCORES)]
    return np.ascontiguousarray(np.concatenate(ys, axis=1).T).astype(np.float32)


# revision 21
# speedup vs baseline: 1.0891x; 1.0891x over previous
"""Trainium2 Bass kernel for nn_MCGRU (per-lab GRU over labs, batch-sharded 8 ways).

Math (per reference):
  demo = static @ demo_W.T + demo_b                      [bs, HID]
  xp   = x @ lab_W.T + lab_b                             [bs, T, LAB]
  per-lab GRU over T steps with input size 1, hidden F:
    gi = xp_t[:,:,None]*Wih + bih ; gh = einsum(h,Whh) + bhh
    r = sig(gi_r+gh_r); z = sig(gi_z+gh_z); n = tanh(gi_n + r*gh_n)
    h' = (1-z)*n + z*h
  out = cat(demo, h_T.reshape) @ out_W.T + out_b         [bs, HID]

Device layout (per core, bs_loc=128 batch rows):
  - GRU state h kept as [(lab,f) partitions, batch free]; labs split into two
    groups of 32 => two independent [128,128] chains per core.
  - Gate pre-activations by block-diagonal matmuls; biases applied by a
    leading K=2 selector matmul per psum tile (lab_b folded into gate biases).
  - sigmoid/tanh on ScalarE, gate algebra on VectorE.
All host-side prep is layout-only (transpose/pack/fold of weights).
"""

import ml_dtypes
import numpy as np

BF16 = ml_dtypes.bfloat16
BS, T, LAB, DEMO, HID, F = 1024, 128, 64, 16, 32, 4
NCORES = 8
BSL = BS // NCORES  # 128 batch rows per core
G = 2               # lab groups per core
LPG = LAB // G      # 32 labs per group
TH = T // 2         # t-half length (xp row-stacking)


def _pack_host(inputs):
    """Layout-only host packing of weights + per-core input shards."""
    x = np.asarray(inputs["x"], np.float32)
    static = np.asarray(inputs["static"], np.float32)
    demo_W = np.asarray(inputs["demo_W"], np.float32)
    demo_b = np.asarray(inputs["demo_b"], np.float32)
    lab_W = np.asarray(inputs["lab_W"], np.float32)
    lab_b = np.asarray(inputs["lab_b"], np.float32)
    Wih = np.asarray(inputs["Wih"], np.float32)
    bih = np.asarray(inputs["bih"], np.float32)
    Whh = np.asarray(inputs["Whh"], np.float32)
    bhh = np.asarray(inputs["bhh"], np.float32)
    out_W = np.asarray(inputs["out_W"], np.float32)
    out_b = np.asarray(inputs["out_b"], np.float32)

    shared = {}
    # xp matmul: out[j, n] = sum_l lab_W[j, l] * xT[l, n]
    shared["wlab"] = np.ascontiguousarray(lab_W.T)  # [64, 64]

    # Per-group block-diagonal GRU weights.
    for g in range(G):
        labs = range(g * LPG, (g + 1) * LPG)
        whr = np.zeros((128, 128), np.float32)
        whz = np.zeros((128, 128), np.float32)
        whn = np.zeros((128, 128), np.float32)
        wxr = np.zeros((32, 128), np.float32)
        wxz = np.zeros((32, 128), np.float32)
        wxn = np.zeros((32, 128), np.float32)
        brz = np.zeros((2, 128), np.float32)
        bnn = np.zeros((2, 128), np.float32)
        for i, l in enumerate(labs):
            s = slice(i * 4, i * 4 + 4)
            # lhsT[k=(i,f_in), m=(i,f_out)] = Whh[l, f_out, f_in]
            whr[s, s] = Whh[l, 0:4, :].T
            whz[s, s] = Whh[l, 4:8, :].T
            whn[s, s] = Whh[l, 8:12, :].T
            wxr[i, s] = Wih[l, 0:4]
            wxz[i, s] = Wih[l, 4:8]
            wxn[i, s] = Wih[l, 8:12]
            # biases; lab_b folded in (xp is computed without lab_b).
            brz[0, s] = bih[l, 0:4] + bhh[l, 0:4] + Wih[l, 0:4] * lab_b[l]
            brz[1, s] = bih[l, 4:8] + bhh[l, 4:8] + Wih[l, 4:8] * lab_b[l]
            bnn[0, s] = bhh[l, 8:12]
            bnn[1, s] = bih[l, 8:12] + Wih[l, 8:12] * lab_b[l]
        shared[f"whr{g}"] = whr
        shared[f"whz{g}"] = whz
        shared[f"whn{g}"] = whn
        shared[f"brz{g}"] = brz
        shared[f"bnn{g}"] = bnn
        shared[f"_wxr{g}"] = wxr
        shared[f"_wxz{g}"] = wxz
        shared[f"_wxn{g}"] = wxn

    # x-side weights stacked so lhsT slices share the xpa base partition:
    # rows [g*32 : g*32+32] hold group g.
    for nm in ("wxr", "wxz", "wxn"):
        wall = np.zeros((64, 128), np.float32)
        for g in range(G):
            wall[g * 32:(g + 1) * 32, :] = shared[f"_{nm}{g}"]
        shared[f"{nm}a"] = wall
    for g in range(G):
        del shared[f"_wxr{g}"], shared[f"_wxz{g}"], shared[f"_wxn{g}"]

    # selector rows for the bias matmuls: row0 -> first BSL cols, row1 -> rest
    sel2 = np.zeros((2, 2 * BSL), np.float32)
    sel2[0, :BSL] = 1.0
    sel2[1, BSL:] = 1.0
    shared["sel2"] = sel2

    # Output layer. feat index (l, f) -> col HID + l*4 + f of out_W.
    w_feat = out_W[:, HID:]  # [32, 256]
    for g in range(G):
        wo = np.zeros((128, HID), np.float32)
        for i, l in enumerate(range(g * LPG, (g + 1) * LPG)):
            wo[i * 4:(i + 1) * 4, :] = w_feat[:, l * 4:(l + 1) * 4].T
        shared[f"wout{g}"] = wo
    shared["woutd"] = np.ascontiguousarray(out_W[:, :HID].T)  # [32, 32]
    shared["woutb"] = out_b.reshape(1, HID).copy()            # [1, 32]
    # demo matmul lhsT: [17, 32]; row 0 = demo_b (ones row of statt is row 0)
    wdemo = np.zeros((DEMO + 1, HID), np.float32)
    wdemo[0, :] = demo_b
    wdemo[1:, :] = demo_W.T
    shared["wdemo"] = wdemo

    # Per-core shards. xs [64, T*BSL], col = t*BSL + b.
    xT = np.ascontiguousarray(x.transpose(2, 1, 0))  # [LAB, T, BS]
    in_maps = []
    for c in range(NCORES):
        m = dict(shared)
        xc = xT[:, :, c * BSL:(c + 1) * BSL]  # [64, 128, 128]
        m["xs"] = np.ascontiguousarray(xc.reshape(LAB, T * BSL))
        st = np.ones((DEMO + 1, BSL), np.float32)
        st[1:, :] = static[c * BSL:(c + 1) * BSL, :].T
        m["statt"] = st
        in_maps.append(m)
    # bf16 for matmul operands (PSUM still accumulates fp32)
    bf_names = {"wlab", "wxra", "wxza", "wxna", "sel2"}
    for g in range(G):
        bf_names |= {f"whr{g}", f"whz{g}", f"whn{g}", f"brz{g}", f"bnn{g}",
                     f"wout{g}"}
    for m in in_maps:
        for n in list(m):
            if n in bf_names or n == "xs":
                m[n] = m[n].astype(BF16)
        m["woutd"] = m["woutd"].astype(BF16)
    return in_maps


def _build_kernel():
    import concourse.bacc as bacc
    import concourse.tile as tile
    from concourse import mybir
    from concourse._compat import get_trn_type

    f32 = mybir.dt.float32
    bf16 = mybir.dt.bfloat16
    nc = bacc.Bacc(get_trn_type() or "TRN2", target_bir_lowering=False, debug=False)

    # DRAM tensors
    d_xs = nc.dram_tensor("xs", (LAB, T * BSL), bf16, kind="ExternalInput")
    d_st = nc.dram_tensor("statt", (DEMO + 1, BSL), f32, kind="ExternalInput")
    wnames = ["wlab", "sel2", "woutd", "woutb", "wdemo", "wxra", "wxza", "wxna"]
    for g in range(G):
        wnames += [f"whr{g}", f"whz{g}", f"whn{g}", f"brz{g}", f"bnn{g}",
                   f"wout{g}"]
    wshapes = {
        "wlab": (LAB, LAB), "sel2": (2, 2 * BSL), "woutd": (HID, HID),
        "woutb": (1, HID), "wdemo": (DEMO + 1, HID),
        "wxra": (64, 128), "wxza": (64, 128), "wxna": (64, 128),
    }
    for g in range(G):
        wshapes.update({
            f"whr{g}": (128, 128), f"whz{g}": (128, 128), f"whn{g}": (128, 128),
            f"brz{g}": (2, 128), f"bnn{g}": (2, 128), f"wout{g}": (128, HID),
        })
    bf_set = {"wlab", "wxra", "wxza", "wxna", "sel2", "woutd"}
    for g in range(G):
        bf_set |= {f"whr{g}", f"whz{g}", f"whn{g}", f"brz{g}", f"bnn{g}",
                   f"wout{g}"}
    dws = {n: nc.dram_tensor(n, wshapes[n], bf16 if n in bf_set else f32,
                             kind="ExternalInput")
           for n in wnames}
    d_y = nc.dram_tensor("y", (HID, BSL), f32, kind="ExternalOutput")

    Sig = mybir.ActivationFunctionType.Sigmoid
    Tanh = mybir.ActivationFunctionType.Tanh

    with tile.TileContext(nc) as tc:
        with (
            tc.tile_pool(name="const", bufs=1) as cpool,
            tc.tile_pool(name="xp", bufs=1) as xpool,
            tc.tile_pool(name="state", bufs=3) as spool,
            tc.tile_pool(name="work", bufs=4) as wpool,
        ):
            # ---- load weights (small, SWDGE via gpsimd) ----
            wt = {}
            for name in wnames + ["statt"]:
                dt_ = dws[name] if name != "statt" else d_st
                t_ = cpool.tile(list(dt_.shape), dt_.dtype, tag=name)
                nc.gpsimd.dma_start(t_[:], dt_[:])
                wt[name] = t_

            # xp tiles (raw xp, no lab_b): rows = labs, col = t*BSL + b,
            # quartered over t so the scan can start before phase 1 ends.
            QT = T // 4
            xp_q = [xpool.tile([LAB, QT * BSL], bf16, tag=f"xp_sb{q}",
                               name=f"xp_sb{q}")
                    for q in range(4)]

            # ---- phase 1: xp = lab_W @ x (bias folded into gate biases) ----
            with (
                tc.tile_pool(name="xsb", bufs=1) as xsbp,
                tc.tile_pool(name="p1", bufs=3, space="PSUM") as p1pool,
            ):
                xs_q = [xsbp.tile([LAB, T * BSL // 4], bf16, tag=f"xs{q}",
                                  name=f"xs{q}")
                        for q in range(4)]
                for q in range(4):
                    half = T * BSL // 8
                    for j in range(2):
                        cs = slice(j * half, (j + 1) * half)
                        nc.sync.dma_start(xs_q[q][:, cs],
                                          d_xs[:, q * 2 * half + j * half:
                                               q * 2 * half + (j + 1) * half])
                NCH = T * BSL // 512  # 32 chunks of 512
                for i in range(NCH):
                    q, iq = divmod(i, NCH // 4)
                    cs = slice(iq * 512, (iq + 1) * 512)
                    ps = p1pool.tile([LAB, 512], f32, tag="xpp")
                    nc.tensor.matmul(ps[:], wt["wlab"][:], xs_q[q][:, cs],
                                     start=True, stop=True)
                    if i % 2 == 0:
                        nc.vector.tensor_copy(xp_q[q][:, cs], ps[:])
                    else:
                        nc.scalar.copy(xp_q[q][:, cs], ps[:])

            # ---- demo head (independent of scan) ----
            with tc.tile_pool(name="pd", bufs=1, space="PSUM") as pdpool:
                ps_d = pdpool.tile([HID, BSL], f32, tag="psd")
                nc.tensor.matmul(ps_d[:], wt["wdemo"][:], wt["statt"][:],
                                 start=True, stop=True)
                demo_sb = cpool.tile([HID, BSL], bf16, tag="demo_sb")
                nc.vector.tensor_copy(demo_sb[:], ps_d[:])

            # ---- phase 2: GRU scan ----
            h = []
            for g in range(G):
                hg = spool.tile([128, BSL], bf16, tag=f"h{g}")
                nc.gpsimd.memset(hg[:], 0.0)
                h.append(hg)

            with (
                tc.tile_pool(name="prz", bufs=2, space="PSUM") as przp,
                tc.tile_pool(name="pnn", bufs=2, space="PSUM") as pnnp,
            ):
                for t in range(T):
                    q, tq = divmod(t, T // 4)
                    rzs_l, nn_l, tt_l, uu_l, nt_l, zh_l = {}, {}, {}, {}, {}, {}
                    for g in range(G):
                        rsl = slice(g * 32, (g + 1) * 32)
                        xpa = xp_q[q][rsl, tq * BSL:(tq + 1) * BSL]
                        rz = przp.tile([128, 2 * BSL], f32, tag=f"rz{g}")
                        nn = pnnp.tile([128, 2 * BSL], f32, tag=f"nn{g}")
                        nn_l[g] = nn
                        # Region runs must be consecutive and never revisit
                        # a psum region (HW accumulation constraint).
                        nc.tensor.matmul(rz[:], wt[f"brz{g}"][:], wt["sel2"][:],
                                         start=True, stop=False)
                        nc.tensor.matmul(rz[:, 0:BSL], wt[f"whr{g}"][:], h[g][:],
                                         start=False, stop=False)
                        nc.tensor.matmul(rz[:, 0:BSL], wt["wxra"][rsl, :], xpa,
                                         start=False, stop=False)
                        nc.tensor.matmul(rz[:, BSL:], wt[f"whz{g}"][:], h[g][:],
                                         start=False, stop=False)
                        nc.tensor.matmul(rz[:, BSL:], wt["wxza"][rsl, :], xpa,
                                         start=False, stop=True)
                        nc.tensor.matmul(nn[:], wt[f"bnn{g}"][:], wt["sel2"][:],
                                         start=True, stop=False)
                        nc.tensor.matmul(nn[:, 0:BSL], wt[f"whn{g}"][:], h[g][:],
                                         start=False, stop=False)
                        nc.tensor.matmul(nn[:, BSL:], wt["wxna"][rsl, :], xpa,
                                         start=False, stop=True)
                        # sigmoid split per gate: r is on the critical path.
                        rzs = wpool.tile([128, 2 * BSL], bf16, tag=f"rzs{g}")
                        rzs_l[g] = rzs
                        nc.scalar.activation(rzs[:, 0:BSL], rz[:, 0:BSL], Sig)
                        tt = wpool.tile([128, BSL], bf16, tag=f"tt{g}")
                        tt_l[g] = tt
                        nc.vector.tensor_mul(tt[:], rzs[:, 0:BSL], nn[:, 0:BSL])
                        nc.scalar.activation(rzs[:, BSL:], rz[:, BSL:], Sig)
                    for g in range(G):
                        rzs, nn, tt = rzs_l[g], nn_l[g], tt_l[g]
                        uu = wpool.tile([128, BSL], f32, tag=f"uu{g}")
                        nc.vector.tensor_add(uu[:], tt[:], nn[:, BSL:])
                        zh = wpool.tile([128, BSL], bf16, tag=f"zh{g}")
                        nc.vector.tensor_mul(zh[:], rzs[:, BSL:], h[g][:])
                        zh_l[g] = zh
                        nt = wpool.tile([128, BSL], bf16, tag=f"nt{g}")
                        nt_l[g] = nt
                        nc.scalar.activation(nt[:], uu[:], Tanh)
                    for g in range(G):
                        rzs, nt, zh = rzs_l[g], nt_l[g], zh_l[g]
                        # h' = z*h + (1-z)*n = zh - (z-1)*n
                        aa = wpool.tile([128, BSL], bf16, tag=f"aa{g}")
                        nc.vector.scalar_tensor_tensor(
                            aa[:], rzs[:, BSL:], 1.0, nt[:],
                            mybir.AluOpType.subtract, mybir.AluOpType.mult)
                        hn = spool.tile([128, BSL], bf16, tag=f"h{g}")
                        nc.vector.tensor_sub(hn[:], zh[:], aa[:])
                        h[g] = hn

            # ---- phase 3: output head ----
            with tc.tile_pool(name="po", bufs=1, space="PSUM") as popool:
                ps_o = popool.tile([HID, BSL], f32, tag="pso")
                nc.tensor.matmul(ps_o[:], wt["wout0"][:], h[0][:],
                                 start=True, stop=False)
                nc.tensor.matmul(ps_o[:], wt["wout1"][:], h[1][:],
                                 start=False, stop=False)
                nc.tensor.matmul(ps_o[:], wt["woutd"][:], demo_sb[:],
                                 start=False, stop=False)
                nc.tensor.matmul(ps_o[:], wt["woutb"][:],
                                 wt["statt"][0:1, :],
                                 start=False, stop=True)
                y_sb = cpool.tile([HID, BSL], f32, tag="y_sb")
                nc.vector.tensor_copy(y_sb[:], ps_o[:])
                nc.sync.dma_start(d_y[:], y_sb[:])

    nc.compile()
    return nc


_NC_CACHE = None


def _get_nc():
    global _NC_CACHE
    if _NC_CACHE is None:
        _NC_CACHE = _build_kernel()
    return _NC_CACHE


def kernel(**inputs):
    from concourse import bass_utils

    in_maps = _pack_host(inputs)
    nc = _get_nc()
    res = bass_utils.run_bass_kernel_spmd(nc, in_maps, list(range(NCORES)))
    ys = [np.asarray(res.results[c]["y"]) for c in range(NCORES)]
    return np.ascontiguousarray(np.concatenate(ys, axis=1).T).astype(np.float32)
